# revision 29
# baseline (speedup 1.0000x reference)
"""FJSP decoder kernel for Trainium2, data-parallel over batch on 8 NeuronCores.

Same factorized-attention algebra as before (scores over the joint (job,
machine) axis decompose as E + F outer sums, so the softmax splits into
per-head [100,20]-shaped matmul work), but restructured for latency:

- All weight massaging is folded on the host: q/k weights arrive pre-padded
  into the 32-lane-per-head layout (zeros interleaved), v is folded through
  Wmhc@Wshc into per-head uv weights (Wv[:,h] @ w2_h), and the scalar output
  bias is pre-broadcast. The device issues two DMAs (bf16 weights+acts, fp32
  mask+bias) and starts projecting immediately - no on-device transposes,
  no identity, no wpad construction.
- Everything upstream of PSUM is bf16 (1 cycle/row on PE instead of fp32's
  4), sized so the rel-err stays ~8e-3 against the fp32 reference.
- Heads are processed in pairs: one [100,480] PSUM bank holds both heads'
  score blocks, one exp per pair (fixed activation overhead amortized).
- The combine is 3 DVE ops (two strided divides + one reduce) instead of
  recip/mul chains, and the tanh uses the Tanh table directly.
"""

import math

import numpy as np
import ml_dtypes

import concourse.bass as bass
import concourse.mybir as mybir
import concourse.tile as tile
from concourse.bass_utils import run_bass_kernel_spmd

F32 = mybir.dt.float32
BF16 = mybir.dt.bfloat16
AF = mybir.ActivationFunctionType
OP = mybir.AluOpType
AX = mybir.AxisListType

D, H, QD = 128, 8, 16
B, J, M = 8, 100, 20
INV_SQ = 1.0 / math.sqrt(QD)  # 0.25
SD = math.sqrt(D)

# wb (bf16) column layout: [ejT 0:100 | emT 100:120 | pad 120:128 |
#   8 wpad tiles 128:1152 | Wuvj 1152:1160 | Wuvm 1160:1168 |
#   raw-f32-bits: mask 1168:1208 (=20 f32) | bias 1208:1210 (=1 f32) | pad]
EJ0, EM0, WT0, UVJ0, UVM0 = 0, 100, 128, 1152, 1160
MK0, BIAS0 = 1168, 1208
WB_W = 1212
# wpad tile order: grp-major so wb1 carries everything grp0 needs
WTILE = {}
for _i, (_g, _nm, _h) in enumerate(
    [(g, n, h) for g in (0, 1) for n in ("q", "k") for h in ("j", "m")]
):
    WTILE[(_nm, _h, _g)] = WT0 + 128 * _i
WB_SPLIT1 = 384  # wb1 = ejT|emT + q-grp0 tiles
WB_SPLIT2 = 640  # wb2 = k-grp0 tiles; wb3 = the rest

_PATCHED = False


def _install_drain_patch():
    # gen3 walrus accepts one sync-wait per instruction. Tile's kernel-tail
    # drain accumulates one wait per active logical processor on a single
    # Drain: spread them across engines (parallel waiting).
    global _PATCHED
    if _PATCHED:
        return
    from concourse.tile import ScopedClock, TileContext

    def _split_drain_and_barrier(self, tick_clock, wait_clock):
        drain_inst = self.nc.sync.drain()
        wait_clock.add_sem_waits(
            drain_inst.ins, ScopedClock({None: tick_clock.global_clock})
        )
        si = drain_inst.ins.sync_info
        waits = list(si.on_wait) if si is not None else []
        if len(waits) > 1:
            assert not si.on_update
            sems = {s.name: s for s in self.sems.allocated().values()}
            drain_inst.ins.sync_info = None
            drain_inst.wait_op(sems[waits[0].ant_name], waits[0].wait_value, "sem-ge")
            engines = [
                self.nc.scalar,
                self.nc.vector,
                self.nc.tensor,
                self.nc.gpsimd,
                self.nc.sync,
            ]
            for i, w in enumerate(waits[1:]):
                extra = engines[i % len(engines)].drain()
                extra.wait_op(sems[w.ant_name], w.wait_value, "sem-ge")
        self.nc.all_engine_barrier()
        assert self.sems is not None
        popped = self.nc._tile_sem_poison_stack.pop()
        assert popped is self._sem_poison
        self.nc.clear_and_free_semaphores(list(self.sems.allocated().values()))

    TileContext._drain_and_barrier = _split_drain_and_barrier
    _PATCHED = True


def _split_multi_waits(nc):
    import bass_rust

    ctr = 0
    for fn in nc.m.functions:
        for bb in fn.blocks:
            il = bb.instructions
            if not any(
                i.sync_info is not None and len(i.sync_info.on_wait) > 1 for i in il
            ):
                continue
            new = []
            for ins in il:
                si = ins.sync_info
                if si is not None and len(si.on_wait) > 1:
                    waits = list(si.on_wait)
                    ups = list(si.on_update)
                    for w in waits[:-1]:
                        nop = mybir.InstNoOp(name=f"I-waitsplit-{ctr}", ins=[], outs=[])
                        ctr += 1
                        nop.engine = ins.engine
                        nop.sync_info = bass_rust.SyncInfo(on_update=[], on_wait=[w])
                        new.append(nop)
                    ins.sync_info = bass_rust.SyncInfo(
                        on_update=ups, on_wait=[waits[-1]]
                    )
                new.append(ins)
            bb.instructions = new


def _fix_prep_dma_sem(nc):
    """Point the SWDGE prep's DMA-completion update (on_update[0]) at the
    Tile-owned DMASW lane sem the tail drain actually waits on. Tile books
    the prep on a DMASW proc but leaves the user sem= in slot 0."""
    import copy

    import bass_rust

    dmasw = {}
    preps = []
    for fn in nc.m.functions:
        for bb in fn.blocks:
            for ins in bb.instructions:
                si = ins.sync_info
                if si is None:
                    continue
                for w in si.on_wait:
                    nm = w.ant_name or ""
                    if nm.startswith("DMASW"):
                        dmasw[nm] = (w.id, w.wait_value)
                if isinstance(ins, mybir.InstKVWritebackAnt) and ins.gen_mode == 1:
                    preps.append(ins)
    assert len(preps) == 1 and len(dmasw) == 1, (preps, dmasw)
    (nm, (sid, _)), ins = next(iter(dmasw.items())), preps[0]
    si = ins.sync_info
    ups = list(si.on_update)
    u0 = copy.copy(ups[0])
    u0.id, u0.ant_name = sid, nm
    # Defer the prep's data-input waits to the trigger (desc-gen reads only
    # addresses; the DMA reads the data when the trigger fires).
    prep_waits = list(si.on_wait)
    ins.sync_info = bass_rust.SyncInfo(on_update=[u0] + ups[1:], on_wait=[])
    trig = None
    for fn in nc.m.functions:
        for bb in fn.blocks:
            for i2 in bb.instructions:
                if isinstance(i2, bass_isa_trigger_types()):
                    trig = i2
    tsi = trig.sync_info
    trig.sync_info = bass_rust.SyncInfo(
        on_update=list(tsi.on_update) if tsi else [],
        on_wait=(list(tsi.on_wait) if tsi else []) + prep_waits,
    )


def bass_isa_trigger_types():
    from concourse import bass_isa

    return bass_isa.InstTriggerDma


def _chunk2(ap_slice, chunk_step):
    """Matmul rhs built from two equal column chunks `chunk_step` apart."""
    return bass.AP(
        tensor=ap_slice.tensor,
        offset=ap_slice.offset,
        ap=[ap_slice.ap[0], [chunk_step, 2], ap_slice.ap[1]],
    )


def _hoist_preamble(nc):
    """Move the (dependency-free) input DMAs ahead of the framework's init
    barrier on the SP stream, and spread the const-tensor memsets across
    Pool/DVE so the init barrier closes sooner."""
    fn = nc.m.functions[0]
    blocks = list(fn.blocks)
    dmas = []
    for bb in blocks:
        il = list(bb.instructions)
        mine = [
            i
            for i in il
            if isinstance(i, mybir.InstDMACopy)
            and i.engine == mybir.EngineType.SP
            and not (i.sync_info and i.sync_info.on_wait)
        ]
        if mine:
            dmas += mine
            bb.instructions = [i for i in il if i not in mine]
    assert dmas, "input DMAs not found"
    for bb in blocks:
        il = list(bb.instructions)
        sp_drains = [
            i
            for i in il
            if isinstance(i, mybir.InstDrain) and i.engine == mybir.EngineType.SP
        ]
        if not sp_drains:
            continue
        consts = [
            i
            for i in il
            if isinstance(i, mybir.InstMemset)
            and i.engine == mybir.EngineType.Pool
            and (i.outs and "const-" in str(getattr(i.outs[0], "memref", "")))
        ]
        for i, m in enumerate(consts):
            if i % 2 == 1:
                m.engine = mybir.EngineType.DVE
        pos = il.index(sp_drains[0])
        bb.instructions = il[:pos] + dmas + il[pos:]
        break


def _build():
    nc = bass.Bass()
    wb1_d = nc.dram_tensor("wb1", [D, WB_SPLIT1], BF16, kind="ExternalInput")
    wb2_d = nc.dram_tensor(
        "wb2", [D, WB_SPLIT2 - WB_SPLIT1], BF16, kind="ExternalInput"
    )
    wb3_d = nc.dram_tensor("wb3", [D, WB_W - WB_SPLIT2], BF16, kind="ExternalInput")
    # output exposed in kv_writeback's [batch, dhi, dho, n_ctx] layout
    out_d = nc.dram_tensor("out", [1, D, 1, M], F32, kind="ExternalOutput")

    with tile.TileContext(nc) as tc:
        with (
            tc.tile_pool(name="persist", bufs=1) as pp,
            tc.tile_pool(name="erot", bufs=3) as ep,
            tc.tile_pool(name="ps_p", bufs=1, space="PSUM") as ps_p,
            tc.tile_pool(name="ps_s", bufs=2, space="PSUM") as ps_s,
            tc.tile_pool(name="ps_sf", bufs=1, space="PSUM") as ps_sf,
        ):
            # ---- input DMAs (hoisted pre-barrier post-build) ------------
            wb_sb = pp.tile([D, WB_W], BF16, tag="wb")
            nc.sync.dma_start(out=wb_sb[:, 0:WB_SPLIT1], in_=wb1_d[:])
            nc.sync.dma_start(out=wb_sb[:, WB_SPLIT1:WB_SPLIT2], in_=wb2_d[:])
            nc.sync.dma_start(out=wb_sb[:, WB_SPLIT2:WB_W], in_=wb3_d[:])

            ejT = wb_sb[:, EJ0 : EJ0 + J]
            emT = wb_sb[:, EM0 : EM0 + M]
            emask_v = wb_sb[0:J, MK0 : MK0 + 2 * M].bitcast(F32)
            bias_v = wb_sb[0:J, BIAS0 : BIAS0 + 2].bitcast(F32)

            # ---- Pool prologue: output-DMA descriptor prep + constants --
            # (runs during the input DMA's dead time)
            out_t = pp.tile([D, M], F32, tag="outt")
            nc.gpsimd.memset(out_t[J:D, :], 0.0)
            idx_sb = pp.tile([D, 1], mybir.dt.int32, tag="idx")
            nc.gpsimd.memset(idx_sb, 0)
            ones_sb = pp.tile([J, J], F32, tag="ones")
            nc.gpsimd.memset(ones_sb, 1.0)

            # ---- projections (PE): one PSUM bank per projection ---------
            def wt(nm, half, grp):
                c = WTILE[(nm, half, grp)]
                return wb_sb[:, c : c + 128]

            # uv results live in spare columns of f_ps (bank budget is 8)
            s_ps = ps_sf.tile([D, 8, 40], F32, tag="s")
            f_ps = ps_sf.tile([D, 344], F32, tag="f")
            f3 = f_ps[:, 0:320].rearrange("p (h m) -> p h m", m=40)

            qk_sb = {}
            psP = {}

            def proj(nm, grp, tg):
                ps = ps_p.tile([D, 120], F32, tag=tg, name=tg)
                nc.tensor.matmul(out=ps[:, 0:J], lhsT=wt(nm, "j", grp), rhs=ejT)
                nc.tensor.matmul(out=ps[:, J : J + M], lhsT=wt(nm, "m", grp), rhs=emT)
                psP[(nm, grp)] = ps

            proj("q", 0, "p0")
            proj("k", 0, "p1")
            nc.tensor.matmul(
                out=f_ps[0:J, 320:328], lhsT=ejT, rhs=wb_sb[:, UVJ0 : UVJ0 + 8]
            )
            nc.tensor.matmul(
                out=f_ps[0:M, 328:336], lhsT=emT, rhs=wb_sb[:, UVM0 : UVM0 + 8]
            )
            qk_sb[("q", 0)] = pp.tile([D, 120], BF16, tag="qt0", name="qt0")
            nc.scalar.copy(out=qk_sb[("q", 0)], in_=psP[("q", 0)])
            qk_sb[("k", 0)] = pp.tile([D, 120], BF16, tag="kt0", name="kt0")
            nc.vector.tensor_copy(out=qk_sb[("k", 0)], in_=psP[("k", 0)])
            uv_sb = pp.tile([J, 16], F32, tag="uv")
            nc.vector.tensor_copy(out=uv_sb, in_=f_ps[0:J, 320:336])
            proj("q", 1, "p2")
            qk_sb[("q", 1)] = pp.tile([D, 120], BF16, tag="qt1", name="qt1")
            nc.vector.tensor_copy(out=qk_sb[("q", 1)], in_=psP[("q", 1)])
            proj("k", 1, "p3")
            qk_sb[("k", 1)] = pp.tile([D, 120], BF16, tag="kt1", name="kt1")
            nc.gpsimd.tensor_copy(out=qk_sb[("k", 1)], in_=psP[("k", 1)])
            c8 = pp.tile([D, M, 16], F32, tag="c8")

            def head_pair(p):
                grp = p // 2
                qt, kt = qk_sb[("q", grp)], qk_sb[("k", grp)]
                ps = ps_s.tile([D, 480], F32, tag="sc")
                for i, h in enumerate((2 * p, 2 * p + 1)):
                    g = h % 4
                    st = slice(32 * g, 32 * g + 32)
                    tp = (32 * g, 0)
                    o = 240 * i
                    nc.tensor.matmul(
                        out=ps[0:J, o : o + 120],
                        lhsT=kt[st, 0:J],
                        rhs=qt[st, 0:120],
                        tile_position=tp,
                    )
                    nc.tensor.matmul(
                        out=ps[0:M, o + 120 : o + 240],
                        lhsT=kt[st, J : J + M],
                        rhs=qt[st, 0:120],
                        tile_position=tp,
                    )
                e1 = ep.tile([D, 560], BF16, tag="e1")
                nc.scalar.activation(
                    out=e1[0:J, 0:480], in_=ps[0:J, 0:480], func=AF.Exp, scale=INV_SQ
                )
                for i, h in enumerate((2 * p, 2 * p + 1)):
                    o = 240 * i
                    so = 480 + 40 * i
                    nc.vector.tensor_scalar_mul(
                        out=e1[0:J, so : so + 20],
                        in0=e1[0:J, o + 100 : o + 120],
                        scalar1=uv_sb[:, h : h + 1],
                    )
                    nc.vector.tensor_scalar_mul(
                        out=e1[0:M, so + 20 : so + 40],
                        in0=e1[0:M, o + 220 : o + 240],
                        scalar1=uv_sb[0:M, 8 + h : 9 + h],
                    )
                for i, h in enumerate((2 * p, 2 * p + 1)):
                    o = 240 * i
                    so = 480 + 40 * i
                    nc.tensor.matmul(
                        out=s_ps[0:J, h, :],
                        lhsT=e1[0:J, o : o + J],
                        rhs=_chunk2(e1[0:J, o + 100 : o + 120], so - o - 100),
                    )
                    nc.tensor.matmul(
                        out=f3[0:J, h, :],
                        lhsT=e1[0:M, o + 120 : o + 220],
                        rhs=_chunk2(e1[0:M, o + 220 : o + 240], so - o - 200),
                    )

            def divides(lo, n, s_eng=None):
                # heads lo..lo+n: Nj/SE and Nm/SF divides
                hs = slice(lo, lo + n)
                (s_eng or nc.vector).tensor_tensor(
                    out=c8[0:J, :, lo : lo + n].rearrange("p m h -> p h m"),
                    in0=s_ps[0:J, hs, 20:40],
                    in1=s_ps[0:J, hs, 0:20],
                    op=OP.divide,
                )
                nc.gpsimd.tensor_tensor(
                    out=c8[0:J, :, 8 + lo : 8 + lo + n].rearrange("p m h -> p h m"),
                    in0=f3[0:J, hs, 20:40],
                    in1=f3[0:J, hs, 0:20],
                    op=OP.divide,
                )

            def c8slots(lo, n):
                # c8 slots {lo:lo+n} and {8+lo:8+lo+n} as one strided AP
                return bass.AP(
                    tensor=c8.tensor,
                    offset=c8.offset + lo,
                    ap=[c8.ap[0], [16, M], [8, 2], [1, n]],
                )

            head_pair(0)
            head_pair(1)
            divides(0, 4)
            head_pair(2)
            divides(4, 2, s_eng=nc.gpsimd)
            head_pair(3)
            divides(6, 2)
            c1 = pp.tile([J, M], F32, tag="c1")
            nc.vector.reduce_sum(out=c1, in_=c8[0:J], axis=AX.X)

            # ---- tail: p = softmax(10*tanh(score1) + mask) --------------
            # exp(10*tanh + mask) = exp(10*tanh) * emask  (emask host-packed)
            t_sb = pp.tile([J, M], F32, tag="t")
            nc.scalar.activation(
                out=t_sb, in_=c1, func=AF.Tanh, scale=1.0 / SD, bias=bias_v
            )
            e10 = pp.tile([J, M], F32, tag="e10")
            nc.scalar.activation(out=e10, in_=t_sb, func=AF.Exp, scale=10.0)
            e_sb = pp.tile([J, M], F32, tag="e")
            s_row = pp.tile([J, 1], F32, tag="srow")
            nc.vector.tensor_tensor_reduce(
                out=e_sb,
                in0=e10,
                in1=emask_v,
                scale=1.0,
                scalar=0.0,
                op0=OP.mult,
                op1=OP.add,
                accum_out=s_row,
            )
            tot_ps = ps_s.tile([D, 480], F32, tag="sc")
            nc.tensor.matmul(out=tot_ps[0:J, 0:1], lhsT=ones_sb, rhs=s_row)
            nc.vector.scalar_tensor_tensor(
                out=out_t[0:J, :],
                in0=e10,
                scalar=tot_ps[0:J, 0:1],
                in1=emask_v,
                op0=OP.divide,
                op1=OP.mult,
            )
            # prep issued after the out_t write: its deferred read gives the
            # prep no sync dep (desc-gen overlaps the tail), the RAW edge
            # lands on the trigger.
            dma_sem = nc.alloc_semaphore("out_dma")
            nc.gpsimd.kv_writeback(
                out_d[:],
                out_t.rearrange("p (a b m) -> p a b m", a=1, b=1),
                idx_sb,
                prepare_only=True,
                sem=dma_sem,
            )
            nc.gpsimd.trigger_dma(count=None)

    _fix_prep_dma_sem(nc)
    _hoist_preamble(nc)
    _split_multi_waits(nc)
    return nc


_NC = None
last_results = None


def _pack_weights(inputs):
    Wq3 = np.asarray(inputs["Wq3"], np.float32)
    Wk = np.asarray(inputs["Wk"], np.float32)
    Wv = np.asarray(inputs["Wv"], np.float32)
    Wmhc = np.asarray(inputs["Wmhc"], np.float32)
    Wshc = np.asarray(inputs["Wshc"], np.float32).reshape(D)
    b_mhc = np.asarray(inputs["b_mhc"], np.float32).reshape(D)
    b_shc = float(np.asarray(inputs["b_shc"]).reshape(-1)[0])

    w2 = Wmhc @ Wshc
    bias_const = float(b_mhc @ Wshc + b_shc)
    w2b = w2.reshape(H, QD)
    Wuvj = np.einsum("dhq,hq->dh", Wv[:D].reshape(D, H, QD), w2b)
    Wuvm = np.einsum("dhq,hq->dh", Wv[D:].reshape(D, H, QD), w2b)

    wb = np.zeros((D, WB_W), ml_dtypes.bfloat16)
    for nm, W in (("q", Wq3), ("k", Wk)):
        for half, Wh in (("j", W[:D]), ("m", W[D:])):
            for grp in range(2):
                c = WTILE[(nm, half, grp)]
                t = wb[:, c : c + 128].reshape(D, 4, 32)
                t[:, :, :16] = Wh[:, 64 * grp : 64 * grp + 64].reshape(D, 4, 16)
    wb[:, UVJ0 : UVJ0 + 8] = Wuvj
    wb[:, UVM0 : UVM0 + 8] = Wuvm
    # bias column as raw f32 bits (device reads it via bitcast)
    f32view = wb[:, BIAS0 : BIAS0 + 2].view(np.uint16).view(np.float32)
    f32view[0:J, 0] = bias_const / SD
    return wb


def kernel(**inputs):
    global _NC, last_results
    _install_drain_patch()
    if _NC is None:
        _NC = _build()

    wb_base = _pack_weights(inputs)
    ejs = np.asarray(inputs["encoded_job"], np.float32)
    ems = np.asarray(inputs["encoded_machine"], np.float32)
    msks = np.asarray(inputs["ninf_mask"], np.float32)

    in_maps = []
    for b in range(B):
        wb = wb_base.copy()
        wb[:, EJ0 : EJ0 + J] = ejs[b].T.astype(ml_dtypes.bfloat16)
        wb[:, EM0 : EM0 + M] = ems[b].T.astype(ml_dtypes.bfloat16)
        mview = wb[:, MK0 : MK0 + 2 * M].view(np.uint16).view(np.float32)
        mview[0:J, 0:M] = np.exp(msks[b])
        in_maps.append(
            {
                "wb1": wb[:, :WB_SPLIT1].copy(),
                "wb2": wb[:, WB_SPLIT1:WB_SPLIT2].copy(),
                "wb3": wb[:, WB_SPLIT2:].copy(),
            }
        )

    last_results = run_bass_kernel_spmd(_NC, in_maps, core_ids=list(range(B)))
    out = np.stack(
        [
            last_results.results[b]["out"].reshape(D, M)[0:J].reshape(J * M)
            for b in range(B)
        ]
    )
    return out.astype(np.float32)
